# revision 15
# baseline (speedup 1.0000x reference)
"""DiT dual-softmax attention on 8 Trainium2 NeuronCores — v2.

Sharding: core c in [0,8) handles (b = c//4, query chunk sc = c%4 of 512).
Each core computes all 16 heads for its 512 queries against the full 2048
keys/values, including the output projection, so the full output is a pure
concatenation — no cross-core collective.

v2 changes vs v1 (409us):
 - natural-layout AV: o[s,66] accumulated with exp tiles as the stationary
   matmul operand and vp streamed (66-col streams; Ldweights pipelines), which
   halves AV matmul cycles and eliminates the o-transposes + PSUM->SBUF copies
 - AV matmuls for head h-1 are interleaved between head h's score matmuls so
   the PE never idles while ACT/DVE consume score tiles
 - the e2 (gain-weighted) softmax branch uses a Schraudolph bf16 exp computed
   on the Vector engine (int16(x*A+B) bitcast to bf16), removing ~40% of the
   Activation-engine exp load; constant factors cancel in softmax
 - head-pair packing (K=128) for the q projections and output projection
 - per-head weight stacks for q-projection are block-diagonal so two heads
   share one matmul
Math per head: s = qh (wq^T wk / sqrt(hd)) kh^T  (host-folded M matrices),
vp_aug = [vh wv^T | rowsum | 1], oX = PX^T-free AV, combine with 1/denom,
groupnorm over hd via bn_stats, out = sum_h o_norm_h @ woT_h rows.
"""
import numpy as np

import concourse.bass as bass
import concourse.mybir as mybir
import concourse.tile as tile
from concourse.masks import make_identity

# ---------------------------------------------------------------------------
# Workaround: this walrus build only accepts 1 semaphore wait per instruction
# (setupSyncWait "Too many sync wait commands"). Post-pass: any instruction
# carrying N>1 waits gets N-1 same-engine NoOp carriers inserted before it.
import bass_rust

_MAX_WAITS = 1
_CARRIER_ID = [0]


def _make_wait_drain(nc, engine, waits):
    _CARRIER_ID[0] += 1
    inst = mybir.InstDrain(name=f"WD-{_CARRIER_ID[0]}", ins=[], outs=[])
    inst.engine = engine
    inst.sync_info = bass_rust.SyncInfo(on_wait=list(waits), on_update=[])
    nc.register_instruction(inst, overwrite=True)
    return inst


def _split_multi_waits(nc):
    f = nc.m.functions[0]
    for b in f.blocks:
        il = b.instructions
        needs = any(
            ins.sync_info is not None and len(ins.sync_info.on_wait) > _MAX_WAITS
            for ins in il
        )
        if not needs:
            continue
        new = []
        for ins in il:
            si = ins.sync_info
            if si is not None and len(si.on_wait) > _MAX_WAITS:
                waits = list(si.on_wait)
                keep = waits[-_MAX_WAITS:]
                carry = waits[:-_MAX_WAITS]
                # A Matmult is always preceded by its own Ldweights (same
                # engine, no consumers between) — hoisting waits there is
                # order-equivalent. Use spare LDW wait slots first.
                if (
                    ins.opcode == "Matmult"
                    and new
                    and new[-1].opcode == "Ldweights"
                    and new[-1].engine == ins.engine
                ):
                    ldw = new[-1]
                    ldw_si = ldw.sync_info
                    ldw_waits = list(ldw_si.on_wait) if ldw_si is not None else []
                    while carry and len(ldw_waits) < _MAX_WAITS:
                        ldw_waits.append(carry.pop())
                    if ldw_si is None:
                        ldw.sync_info = bass_rust.SyncInfo(
                            on_wait=ldw_waits, on_update=[]
                        )
                    else:
                        ldw_si.on_wait = ldw_waits
                    if carry:
                        pos = len(new) - 1
                        carriers = [
                            _make_wait_drain(nc, ins.engine, carry[i : i + _MAX_WAITS])
                            for i in range(0, len(carry), _MAX_WAITS)
                        ]
                        new[pos:pos] = carriers
                else:
                    for i in range(0, len(carry), _MAX_WAITS):
                        new.append(
                            _make_wait_drain(nc, ins.engine, carry[i : i + _MAX_WAITS])
                        )
                si.on_wait = keep
            new.append(ins)
        b.instructions = new
# ---------------------------------------------------------------------------

B, S, D = 2, 2048, 1024
H, HD = 16, 64
NP = H // 2          # head pairs
NS = 512             # queries per core
NC = 8               # cores
EPS = 1e-5
TT = S // 128        # 16 key tiles of 128
ST = NS // 128       # 4 query subtiles of 128
F32 = mybir.dt.float32
BF16 = mybir.dt.bfloat16
I16 = mybir.dt.int16

# Schraudolph bf16 exp: bf16_bits(exp(x)) ~= int16(x * 128/ln2 + 127*128 - C)
SCHRAU_A = float(2**7 / np.log(2))
SCHRAU_B = float(127 * 128 - 5.5)


def build(n_heads=H, n_e2_act=1, reps=1):
    """n_e2_act: per head, this many of the 8 e2 score GROUPS (2 key-tiles
    each) get an exact ACT exp; the rest use the DVE Schraudolph approximation
    (engine load balance)."""
    nc = bass.Bass()
    qt2 = nc.declare_dram_parameter("qt2", [128, NP, NS], BF16, isOutput=False)
    kt2 = nc.declare_dram_parameter("kt2", [128, NP, S], BF16, isOutput=False)
    vt2 = nc.declare_dram_parameter("vt2", [128, NP, S], BF16, isOutput=False)
    m1b = nc.declare_dram_parameter("m1b", [128, NP, 128], BF16, isOutput=False)
    m2b = nc.declare_dram_parameter("m2b", [128, NP, 128], BF16, isOutput=False)
    wvt2 = nc.declare_dram_parameter("wvt2", [128, NP, HD + 1], BF16, isOutput=False)
    wot2 = nc.declare_dram_parameter("wot2", [128, NP, D], BF16, isOutput=False)
    gc = nc.declare_dram_parameter("gc", [128, 1], F32, isOutput=False)
    out = nc.declare_dram_parameter("out", [NS, D], F32, isOutput=True)

    with tile.TileContext(nc) as tc:
        with (
            tc.tile_pool(name="consts", bufs=1) as consts,
            tc.tile_pool(name="io", bufs=2) as io,
            tc.tile_pool(name="epool", bufs=2) as epool,
            tc.tile_pool(name="vpool", bufs=2) as vpool,
            tc.tile_pool(name="work", bufs=2) as work,
            tc.tile_pool(name="outsb", bufs=2) as outsb,
            tc.tile_pool(name="scp", bufs=2, space="PSUM") as scp,
            tc.tile_pool(name="opool", bufs=2, space="PSUM") as opool,
            tc.tile_pool(name="vtp", bufs=2, space="PSUM") as vtp,
        ):
            ident = consts.tile([128, 128], BF16)
            make_identity(nc, ident)
            ident32 = consts.tile([HD + 2, HD + 2], F32)
            make_identity(nc, ident32)
            m1sb = consts.tile([128, NP, 128], BF16)
            nc.sync.dma_start(out=m1sb, in_=m1b[:, :, :])
            m2sb = consts.tile([128, NP, 128], BF16)
            nc.sync.dma_start(out=m2sb, in_=m2b[:, :, :])
            wvtsb = consts.tile([128, NP, HD + 1], BF16)
            nc.sync.dma_start(out=wvtsb, in_=wvt2[:, :, :])
            gcsb = consts.tile([128, 1], F32)
            nc.sync.dma_start(out=gcsb, in_=gc[:, :])
            epssb = consts.tile([128, 1], F32)
            nc.vector.memset(epssb, EPS)
            qt2sb = consts.tile([128, NP, NS], BF16)
            wot2sb = consts.tile([128, NP, D], BF16)
            h1T2 = consts.tile([128, NP, NS], BF16)
            h2T2 = consts.tile([128, NP, NS], BF16)
            onT2 = consts.tile([128, NP, NS], BF16)
            outacc = consts.tile([128, 2 * ST, NS], F32)

            import contextlib

            rep_ctx = tc.For_i(0, reps, 1) if reps > 1 else contextlib.nullcontext()

            with rep_ctx:

                # q projections, head pairs packed on K=128 via block-diagonal
                # folded score matrices
                def emit_qproj(p):
                    hp1 = scp.tile([128, NS], F32, tag="sc")
                    nc.tensor.matmul(
                        hp1, m1sb[:, p, :], qt2sb[:, p, :], start=True, stop=True
                    )
                    nc.scalar.copy(out=h1T2[:, p, :], in_=hp1)
                    hp2 = scp.tile([128, NS], F32, tag="sc")
                    nc.tensor.matmul(
                        hp2, m2sb[:, p, :], qt2sb[:, p, :], start=True, stop=True
                    )
                    nc.scalar.copy(out=h2T2[:, p, :], in_=hp2)

                def emit_outproj(pairs, idxs, first, last):
                    for idx in idxs:
                        oc, st = divmod(idx, ST)
                        opp = vtp.tile([128, NS], F32, tag="vp", name="opp")
                        for i, pr in enumerate(pairs):
                            nc.tensor.matmul(
                                opp,
                                onT2[:, pr, st * 128 : (st + 1) * 128],
                                wot2sb[:, pr, oc * 512 : (oc + 1) * 512],
                                start=(i == 0),
                                stop=(i == len(pairs) - 1),
                            )
                        if first:
                            nc.vector.tensor_copy(outacc[:, idx, :], opp)
                        elif not last:
                            nc.vector.scalar_tensor_tensor(
                                out=outacc[:, idx, :],
                                in0=opp,
                                scalar=0.0,
                                in1=outacc[:, idx, :],
                                op0=mybir.AluOpType.add,
                                op1=mybir.AluOpType.add,
                            )
                        else:
                            osb = outsb.tile([128, NS], F32, tag="ob")
                            nc.vector.scalar_tensor_tensor(
                                out=osb,
                                in0=opp,
                                scalar=0.0,
                                in1=outacc[:, idx, :],
                                op0=mybir.AluOpType.add,
                                op1=mybir.AluOpType.add,
                            )
                            nc.sync.dma_start(
                                out=out[
                                    st * 128 : (st + 1) * 128,
                                    oc * 512 : (oc + 1) * 512,
                                ],
                                in_=osb,
                            )

                def emit_tail(ph, ops):
                    o1p, o2p = ops
                    phodd, pp = ph % 2, ph // 2
                    rec1 = work.tile([128, ST], F32, tag="r1")
                    nc.vector.reciprocal(out=rec1, in_=o1p[:, :, HD + 1])
                    rec2 = work.tile([128, ST], F32, tag="r2")
                    nc.vector.reciprocal(out=rec2, in_=o2p[:, :, HD + 1])
                    rec2g = work.tile([128, ST], F32, tag="r2g")
                    nc.vector.tensor_scalar_mul(out=rec2g, in0=rec2, scalar1=gcsb)

                    ocomb = work.tile([128, ST, HD + 1], F32, tag="oc")
                    for st in range(ST):
                        t1 = work.tile([128, HD + 1], F32, tag="t1")
                        nc.vector.tensor_scalar_mul(
                            out=t1,
                            in0=o1p[:, st, 0 : HD + 1],
                            scalar1=rec1[:, st : st + 1],
                        )
                        nc.vector.scalar_tensor_tensor(
                            out=ocomb[:, st, :],
                            in0=o2p[:, st, 0 : HD + 1],
                            scalar=rec2g[:, st : st + 1],
                            in1=t1,
                            op0=mybir.AluOpType.mult,
                            op1=mybir.AluOpType.add,
                        )

                    mn = work.tile([128, ST], F32, tag="mn")
                    nc.gpsimd.tensor_scalar(
                        out=mn,
                        in0=ocomb[:, :, HD],
                        scalar1=1.0 / HD,
                        scalar2=None,
                        op0=mybir.AluOpType.mult,
                    )
                    sq = work.tile([128, ST, HD], F32, tag="sq")
                    nc.gpsimd.tensor_tensor(
                        out=sq,
                        in0=ocomb[:, :, 0:HD],
                        in1=ocomb[:, :, 0:HD],
                        op=mybir.AluOpType.mult,
                    )
                    ssq = work.tile([128, ST], F32, tag="ssq")
                    nc.vector.tensor_reduce(
                        out=ssq, in_=sq, axis=mybir.AxisListType.X,
                        op=mybir.AluOpType.add,
                    )
                    nm2 = work.tile([128, ST], F32, tag="nm2")
                    nc.gpsimd.tensor_tensor(
                        out=nm2, in0=mn, in1=mn, op=mybir.AluOpType.mult
                    )
                    vv = work.tile([128, ST], F32, tag="vv")
                    nc.gpsimd.tensor_scalar(
                        out=vv,
                        in0=ssq,
                        scalar1=1.0 / HD,
                        scalar2=None,
                        op0=mybir.AluOpType.mult,
                    )
                    var = work.tile([128, ST], F32, tag="var")
                    nc.gpsimd.tensor_tensor(
                        out=var, in0=vv, in1=nm2, op=mybir.AluOpType.subtract
                    )
                    lnv = work.tile([128, ST], F32, tag="lnv")
                    nc.scalar.activation(
                        lnv,
                        var,
                        func=mybir.ActivationFunctionType.Ln,
                        bias=epssb,
                    )
                    rall = work.tile([128, ST], F32, tag="rall")
                    nc.scalar.activation(
                        rall,
                        lnv,
                        func=mybir.ActivationFunctionType.Exp,
                        scale=-0.5,
                    )
                    onsb = work.tile([128, ST, HD], BF16, tag="on")
                    for st in range(ST):
                        nc.gpsimd.tensor_scalar(
                            out=onsb[:, st, :],
                            in0=ocomb[:, st, 0:HD],
                            scalar1=mn[:, st : st + 1],
                            scalar2=rall[:, st : st + 1],
                            op0=mybir.AluOpType.subtract,
                            op1=mybir.AluOpType.mult,
                        )
                    return onsb

                def emit_avtp(o1T, o2T):
                    o1s = work.tile([HD + 2, NS], F32, tag="o1s")
                    nc.scalar.copy(out=o1s, in_=o1T)
                    o2s = work.tile([HD + 2, NS], F32, tag="o2s")
                    nc.vector.tensor_copy(o2s, o2T)
                    o1p = vtp.tile([128, ST, HD + 2], F32, tag="vp", name="o1p")
                    o2p = vtp.tile([128, ST, HD + 2], F32, tag="vp", name="o2p")
                    for st in range(ST):
                        nc.tensor.transpose(
                            o1p[:, st, :], o1s[:, st * 128 : (st + 1) * 128], ident32
                        )
                    for st in range(ST):
                        nc.tensor.transpose(
                            o2p[:, st, :], o2s[:, st * 128 : (st + 1) * 128], ident32
                        )
                    return o1p, o2p

                def emit_tail_tp(ph, onsb):
                    phodd, pp = ph % 2, ph // 2
                    tpp = vtp.tile([HD, ST, 128], BF16, tag="vp")
                    for st in range(ST):
                        nc.tensor.transpose(tpp[:, st, :], onsb[:, st, :], ident)
                    nc.vector.tensor_copy(
                        onT2[
                            64 * phodd : 64 * phodd + 64, pp, :
                        ].rearrange("p (a b) -> p a b", a=ST),
                        tpp,
                    )

                kts = {0: io.tile([128, S], BF16, tag="kt", name="kp0")}
                nc.sync.dma_start(out=kts[0], in_=kt2[:, 0, :])
                vts = {0: io.tile([128, S], BF16, tag="vt", name="vp0")}
                nc.sync.dma_start(out=vts[0], in_=vt2[:, 0, :])
                for pq in range(NP):
                    nc.sync.dma_start(
                        out=qt2sb[:, pq : pq + 1, :], in_=qt2[:, pq : pq + 1, :]
                    )
                emit_qproj(0)
                emit_qproj(1)

                prev = [None]  # (e1, e2b, vpsb) of head h-1
                pend_tp = [None]

                # outproj quarters: pairs {2q,2q+1} ready after tail(4q+3),
                # which is emitted during head 4q+4 -> safe from head 4q+5 on.
                OP_SCHED = {
                    5: (0, [0, 1, 2]), 6: (0, [3, 4, 5]), 7: (0, [6, 7]),
                    9: (1, [0, 1, 2]), 10: (1, [3, 4, 5]), 11: (1, [6, 7]),
                    13: (2, [0, 1, 2]), 14: (2, [3, 4, 5]), 15: (2, [6, 7]),
                }

                for h in range(n_heads):
                    hodd, p = h % 2, h // 2
                    if hodd == 0 and 2 * (p + 1) < n_heads:
                        kts[p + 1] = io.tile([128, S], BF16, tag="kt", name=f"kp{p+1}")
                        nc.sync.dma_start(out=kts[p + 1], in_=kt2[:, p + 1, :])
                        vts[p + 1] = io.tile([128, S], BF16, tag="vt", name=f"vp{p+1}")
                        nc.sync.dma_start(out=vts[p + 1], in_=vt2[:, p + 1, :])
                    if h == 2:
                        nc.sync.dma_start(out=wot2sb, in_=wot2[:, :, :])
                    if hodd == 0 and p + 2 < NP:
                        emit_qproj(p + 2)
                    lo = 64 * hodd
                    kTh = kts[p][lo : lo + 64, :]
                    vTh = vts[p][lo : lo + 64, :]
                    if hodd == 1:
                        kts.pop(p), vts.pop(p)
                    h1T = h1T2[lo : lo + 64, p, :]
                    h2T = h2T2[lo : lo + 64, p, :]

                    # vp_aug [t, 66]: cols 0..63 = vh @ wv^T, 64 = rowsum, 65 = 1
                    # (vp psum groups are interleaved into the score stream)
                    vpsb = vpool.tile([128, TT, HD + 2], BF16, tag="vps")

                    def emit_vp(g):
                        vpp = vtp.tile([128, 4, HD + 2], F32, tag="vp", name="vpp")
                        for j in range(4):
                            t = 4 * g + j
                            nc.tensor.matmul(
                                vpp[:, j, 0 : HD + 1],
                                vTh[:, t * 128 : (t + 1) * 128],
                                wvtsb[lo : lo + 64, p, :],
                                start=True,
                                stop=True,
                            )
                        nc.scalar.copy(
                            out=vpsb[:, 4 * g : 4 * g + 4, 0 : HD + 1],
                            in_=vpp[:, :, 0 : HD + 1],
                        )

                    # scores (groups of 2 key-tiles) + exp for head h, AV for
                    # head h-1 interleaved between the groups
                    e1 = epool.tile([128, TT, NS], BF16, tag="e1")
                    e2i = epool.tile([128, TT, NS], I16, tag="e2")
                    e2b = e2i.bitcast(BF16)
                    if prev[0] is not None:
                        pe1, pe2b, pvp = prev[0]
                        o1T = opool.tile([HD + 2, NS], F32, tag="o")
                        o2T = opool.tile([HD + 2, NS], F32, tag="o")
                    av_ops = [(sx, t) for t in range(TT) for sx in (0, 1)]
                    AV_CHUNK = [4] * 8

                    def emit_av(n):
                        for sx, t in av_ops[:n]:
                            nc.tensor.matmul(
                                (o1T if sx == 0 else o2T),
                                pvp[:, t, :],
                                (pe1 if sx == 0 else pe2b)[:, t, :],
                                start=(t == 0),
                                stop=(t == TT - 1),
                            )
                        del av_ops[:n]

                    EG = 2
                    for g in range(TT // EG):
                        s1p = scp.tile([128, EG, NS], F32, tag="sc")
                        for j in range(EG):
                            t = EG * g + j
                            nc.tensor.matmul(
                                s1p[:, j, :], kTh[:, t * 128 : (t + 1) * 128], h1T,
                                start=True, stop=True,
                            )
                        nc.scalar.activation(
                            e1[:, EG * g : EG * (g + 1), :],
                            s1p,
                            func=mybir.ActivationFunctionType.Exp,
                        )
                        s2p = scp.tile([128, EG, NS], F32, tag="sc")
                        for j in range(EG):
                            t = EG * g + j
                            nc.tensor.matmul(
                                s2p[:, j, :], kTh[:, t * 128 : (t + 1) * 128], h2T,
                                start=True, stop=True,
                            )
                        if g < n_e2_act:
                            nc.scalar.activation(
                                e2b[:, EG * g : EG * (g + 1), :],
                                s2p,
                                func=mybir.ActivationFunctionType.Exp,
                            )
                        else:
                            nc.vector.tensor_scalar(
                                out=e2i[:, EG * g : EG * (g + 1), :],
                                in0=s2p,
                                scalar1=SCHRAU_A,
                                scalar2=SCHRAU_B,
                                op0=mybir.AluOpType.mult,
                                op1=mybir.AluOpType.add,
                            )
                        if g < 4:
                            emit_vp(g)
                        if g == 3:
                            nc.gpsimd.memset(vpsb[:, :, HD + 1 : HD + 2], 1.0)
                        if prev[0] is not None:
                            emit_av(AV_CHUNK[g])
                    if pend_tp[0] is not None:
                        emit_tail_tp(*pend_tp[0])
                        pend_tp[0] = None
                    if prev[0] is not None:
                        pend_tp[0] = (h - 1, emit_tail(h - 1, emit_avtp(o1T, o2T)))
                    prev[0] = (e1, e2b, vpsb)

                    if h in OP_SCHED and n_heads == H:
                        q, idxs = OP_SCHED[h]
                        emit_outproj([2 * q, 2 * q + 1], idxs, q == 0, False)

                # epilogue: AV + tail of the last head, then final quarter
                pe1, pe2b, pvp = prev[0]
                o1T = opool.tile([HD + 2, NS], F32, tag="o")
                o2T = opool.tile([HD + 2, NS], F32, tag="o")
                for t in range(TT):
                    nc.tensor.matmul(
                        o1T, pvp[:, t, :], pe1[:, t, :],
                        start=(t == 0), stop=(t == TT - 1),
                    )
                for t in range(TT):
                    nc.tensor.matmul(
                        o2T, pvp[:, t, :], pe2b[:, t, :],
                        start=(t == 0), stop=(t == TT - 1),
                    )
                if pend_tp[0] is not None:
                    emit_tail_tp(*pend_tp[0])
                    pend_tp[0] = None
                emit_tail_tp(
                    n_heads - 1, emit_tail(n_heads - 1, emit_avtp(o1T, o2T))
                )
                if n_heads == H:
                    emit_outproj([6, 7], list(range(2 * ST)), False, True)
                else:
                    # debug path for small head counts
                    emit_outproj(
                        sorted({hh // 2 for hh in range(n_heads)}),
                        list(range(2 * ST)),
                        True,
                        False,
                    )
                    for idx in range(2 * ST):
                        oc, st = divmod(idx, ST)
                        osb = outsb.tile([128, NS], F32, tag="ob")
                        nc.vector.tensor_copy(osb, outacc[:, idx, :])
                        nc.sync.dma_start(
                            out=out[
                                st * 128 : (st + 1) * 128, oc * 512 : (oc + 1) * 512
                            ],
                            in_=osb,
                        )
    _split_multi_waits(nc)
    return nc


def _to_bf16(a):
    import ml_dtypes

    return np.asarray(a, dtype=np.float32).astype(ml_dtypes.bfloat16)


def prepare_inputs(q, k, v, wq1, wk1, wq2, wk2, wv, wo, gain):
    """Host-side prep: transposes, weight folding, per-core slicing."""
    q = np.asarray(q, np.float32)
    k = np.asarray(k, np.float32)
    v = np.asarray(v, np.float32)
    wq1, wk1 = np.asarray(wq1, np.float32), np.asarray(wk1, np.float32)
    wq2, wk2 = np.asarray(wq2, np.float32), np.asarray(wk2, np.float32)
    wv, wo = np.asarray(wv, np.float32), np.asarray(wo, np.float32)
    gain = float(np.asarray(gain))

    scale = 1.0 / np.sqrt(HD)
    # M[h] = wq^T @ wk / sqrt(hd): s = qh @ M @ kh^T
    M1 = np.einsum("hed,hef->hdf", wq1, wk1) * scale  # [H, d_q, d_k]
    M2 = np.einsum("hed,hef->hdf", wq2, wk2) * scale
    # block-diagonal pair stacks [128, NP, 128]
    m1b = np.zeros((128, NP, 128), np.float32)
    m2b = np.zeros((128, NP, 128), np.float32)
    for p in range(NP):
        m1b[0:64, p, 0:64] = M1[2 * p]
        m1b[64:128, p, 64:128] = M1[2 * p + 1]
        m2b[0:64, p, 0:64] = M2[2 * p]
        m2b[64:128, p, 64:128] = M2[2 * p + 1]
    m1b_dev, m2b_dev = _to_bf16(m1b), _to_bf16(m2b)

    # wv^T per head with rowsum column, pair-stacked: [128, NP, e+1]
    wvT = wv.transpose(0, 2, 1)  # [H, d, e]
    wvt_aug = np.concatenate([wvT, wvT.sum(axis=2, keepdims=True)], axis=2)
    wvt2 = np.concatenate([wvt_aug[0::2], wvt_aug[1::2]], axis=1)  # [NP, 128, e+1]
    wvt2_dev = _to_bf16(wvt2.transpose(1, 0, 2).copy())

    # wo^T row blocks, pair-stacked: [128, NP, D]
    woT = wo.T.reshape(H, HD, D)  # head h rows
    wot2 = np.concatenate([woT[0::2], woT[1::2]], axis=1)  # [NP, 128, D]
    wot2_dev = _to_bf16(wot2.transpose(1, 0, 2).copy())

    gc_dev = np.full((128, 1), -gain, np.float32)

    qT = q.transpose(0, 2, 1)  # [B, D, S]
    kT = k.transpose(0, 2, 1)
    vT = v.transpose(0, 2, 1)

    in_maps = []
    for c in range(NC):
        b, sc = divmod(c, 4)
        qs = qT[b][:, sc * NS : (sc + 1) * NS].reshape(H, HD, NS)
        # pair-stacked queries [128, NP, NS]
        qt2 = np.concatenate([qs[0::2], qs[1::2]], axis=1)  # [NP, 128, NS]
        ks = kT[b].reshape(H, HD, S)
        vs = vT[b].reshape(H, HD, S)
        kt2 = np.concatenate([ks[0::2], ks[1::2]], axis=1)  # [NP, 128, S]
        vt2 = np.concatenate([vs[0::2], vs[1::2]], axis=1)
        in_maps.append(
            {
                "qt2": _to_bf16(qt2.transpose(1, 0, 2).copy()),
                "kt2": _to_bf16(kt2.transpose(1, 0, 2).copy()),
                "vt2": _to_bf16(vt2.transpose(1, 0, 2).copy()),
                "m1b": m1b_dev,
                "m2b": m2b_dev,
                "wvt2": wvt2_dev,
                "wot2": wot2_dev,
                "gc": gc_dev,
            }
        )
    return in_maps


_NC_CACHE = {}


def kernel(q, k, v, wq1, wk1, wq2, wk2, wv, wo, gain):
    from concourse.bass_utils import run_bass_kernel_spmd

    if "nc" not in _NC_CACHE:
        _NC_CACHE["nc"] = build()
    nc = _NC_CACHE["nc"]
    in_maps = prepare_inputs(q, k, v, wq1, wk1, wq2, wk2, wv, wo, gain)
    res = run_bass_kernel_spmd(nc, in_maps, list(range(NC)))
    out = np.empty((B, S, D), np.float32)
    for c in range(NC):
        b, sc = divmod(c, 4)
        out[b, sc * NS : (sc + 1) * NS, :] = res.results[c]["out"]
    return out
